# revision 21
# baseline (speedup 1.0000x reference)
"""Trainium2 Bass kernel for DualHeterogeneousTransformer (returns out[:, 0] only).

Algebraic reduction (reference returns only query row 0):
  q      = (x[:,0,:] + pos_e[0]) @ We_q^T + be_q                      [B,D]
  qk_e   = (q @ We_k) * scale ; qk_r = (q @ Wr_k) * scale             [B,D]
  s_e[b,k] = x[b,k,:].qk_e[b] + P_e[k].qk_e[b] + (q.be_k)*scale   k<64
  s_e[b,64] = P_e[64].qk_e[b] + (q.be_k)*scale    (mask token, P_e[64]=pos_e[64]+mask)
  s_r[b,k] = r[b].qk_r[b] + pos_r[k].qk_r[b] + (q.br_k)*scale
  p = exp(s), Z = sum(p)
  C_e[b,:] = sum_{k<64} p_e[b,k] x[b,k,:]  +  p_e[b,:65] @ P_e
  C_r[b,:] = (sum_k p_r[b,k]) * r[b]  +  p_r @ pos_r[:64]
  out = (C_e @ We_v^T + C_r @ Wr_v^T + sae*be_v + sar*br_v) / Z

Implementation notes (cost-model driven):
  - x and r are converted to bf16 on the host: halves all x DMA bytes.
  - x streams in [128, 8, 512] chunks: most on the SP HWDGE ring (free-standing
    queue), a few via the gpsimd SWDGE ring.
  - dot products: one broadcast tensor_tensor multiply per chunk (DVE 2x mode /
    gpsimd flat) + per-key accumulating reduces split DVE (4x ts-accum) / ACT
    (copy with accum).
  - O-update: diag(p_k) = ident16 * p_k via one tensor_scalar, then PE
    accumulates p_k*x_k into PSUM via diagonal matmuls at ~215ns/key.
  - all transposes via DMA XBAR (dma_start_transpose, bf16) on HWDGE rings.
  - weights/positional tables pre-folded on host (SCALE and We_q folded in),
    shipped as one bf16 blob + tiny f32 blob.
"""

import os
import sys

import numpy as np
import ml_dtypes

for _p in ("/opt/trn_rl_repo", "/root/.axon_site/_ro/trn_rl_repo"):
    if os.path.isdir(_p) and _p not in sys.path:
        sys.path.insert(0, _p)

import concourse.bass as bass
import concourse.bacc as bacc
import concourse.mybir as mybir
from concourse import tile
from concourse.bass_utils import run_bass_kernel_spmd

B, L, D = 2048, 64, 512
NCORES = 8
BS = B // NCORES          # 256 rows per core
P = 128                   # partition tile of batch rows
NT = BS // P              # 2 batch tiles per core
KC = 8                    # keys per streamed x chunk
NCHUNK = L // KC          # 8 chunks per batch tile
DC = D // P               # 4 contraction chunks of 128
SCALE = float(1.0 / np.sqrt(D))
F32 = mybir.dt.float32
BF16 = mybir.dt.bfloat16
ALU = mybir.AluOpType
ACTF = mybir.ActivationFunctionType
AX = mybir.AxisListType
BF = ml_dtypes.bfloat16

# --- tuning knobs -----------------------------------------------------------
# per-tile chunk ring: True -> gpsimd SWDGE, False -> SP HWDGE
POOL_CHUNK = [False, False, False, False, False, False, False, True]
# per-tile chunk mult engine: True -> gpsimd, False -> DVE
POOL_MULT = [False, True, False, True, False, True, False, True]
# per 8 keys: how many reduces go to ACT (rest DVE ts-accum)
ACT_RED_PER_8 = 3
# per 8 keys: how many diag builds go to gpsimd (rest DVE)
POOL_DIAG_PER_8 = 6

# bf16 blob layout: name -> width (free floats per partition)
_B16_FIELDS = [
    # piece A: gates the q-chain start
    ("pe0", D), ("wqk_e", DC * D), ("ue_row", D), ("onescol", P),
    # piece B: rest of the head + stream ident
    ("wqk_r", D * DC), ("ur_row", D), ("peT", DC * (L + 1)), ("prT", DC * L),
    ("vk2", DC * 2), ("bq2row", 2), ("ident", P), ("pr", D),
    # piece C: tail-only
    ("wevT", DC * D), ("wrvT", DC * D), ("pe", D),
    ("bev", D), ("brv", D),
]
_PIECE_A_END = "wqk_r"
_PIECE_B_END = "wevT"

_BF_FIELDS = [("pidx", 1)]


def _offsets(fields):
    out, off = {}, 0
    for name, w in fields:
        out[name] = (off, w)
        off += w
    return out, off


B16_OFF, BLOB16_W = _offsets(_B16_FIELDS)
BF_OFF, BLOBF_W = _offsets(_BF_FIELDS)


def build_nc():
    nc = bacc.Bacc("TRN2", target_bir_lowering=False, debug=False)

    x_d = nc.dram_tensor("x16", [BS, L, D], BF16, kind="ExternalInput")
    r_d = nc.dram_tensor("r16", [BS, D], BF16, kind="ExternalInput")
    b16_d = nc.dram_tensor("c_blob16", [P, BLOB16_W], BF16, kind="ExternalInput")
    bf_d = nc.dram_tensor("c_blobf", [P, BLOBF_W], F32, kind="ExternalInput")
    out_d = nc.dram_tensor("out", [BS, D], F32, kind="ExternalOutput")

    with tile.TileContext(nc) as tc:
        with (
            tc.tile_pool(name="const", bufs=1) as const,
            tc.tile_pool(name="work", bufs=2) as work,
            tc.tile_pool(name="psum", bufs=4, space="PSUM") as psum,
            tc.tile_pool(name="opsum", bufs=2, space="PSUM") as opsum,
        ):
            # PE warmup to ramp the HAM clock before the q-chain matmuls.
            warm = work.tile([P, P], BF16, tag="warm")
            nc.vector.memset(warm[:], 0.0)
            ps_w = psum.tile([P, P], F32, tag="ps")
            for wi in range(10):
                nc.tensor.matmul(ps_w[:], warm[:], warm[:],
                                 start=(wi == 0), stop=(wi == 9))

            # head DMAs: x0 rows (tiny, SP), r rows (SP), blobs (ACT ring)
            x0_tiles, r_tiles = [], []
            for ts in range(NT):
                rows = slice(ts * P, (ts + 1) * P)
                x0_t = work.tile([P, D], BF16, tag="x0")
                nc.sync.dma_start(x0_t[:], x_d[rows, 0, :])
                x0_tiles.append(x0_t)

            b16 = const.tile([P, BLOB16_W], BF16, tag="b16")
            splitA = B16_OFF[_PIECE_A_END][0]
            splitB = B16_OFF[_PIECE_B_END][0]
            nc.scalar.dma_start(b16[:, 0:splitA], b16_d[:, 0:splitA])
            nc.scalar.dma_start(b16[:, splitA:splitB], b16_d[:, splitA:splitB])
            bfc = const.tile([P, BLOBF_W], F32, tag="bf")
            nc.scalar.dma_start(bfc[:], bf_d[:])

            for ts in range(NT):
                rows = slice(ts * P, (ts + 1) * P)
                r_t = work.tile([P, D], BF16, tag="r")
                nc.sync.dma_start(r_t[:], r_d[rows, :])
                r_tiles.append(r_t)

            def b16v(name, *dims):
                off, w = B16_OFF[name]
                v = b16[:, off:off + w]
                if dims:
                    kw = {chr(97 + i): d_ for i, d_ in enumerate(dims)}
                    pat = " ".join(chr(97 + i) for i in range(len(dims)))
                    v = v.rearrange(f"p ({pat}) -> p {pat}", **kw)
                return v

            pe0_b = b16v("pe0")
            wqk_e_sb = b16v("wqk_e", DC, D)
            wqk_r_sb = b16v("wqk_r", DC, D)
            ue_row = b16v("ue_row")
            ur_row = b16v("ur_row")
            onescol = b16v("onescol")
            peT_sb = b16v("peT", DC, L + 1)
            prT_sb = b16v("prT", DC, L)
            vk2_sb = b16v("vk2", DC, 2)
            bq2row = b16v("bq2row")
            ident16 = b16v("ident")
            wevT_sb = b16v("wevT", DC, D)
            wrvT_sb = b16v("wrvT", DC, D)
            pe_sb = b16v("pe")
            pr_sb = b16v("pr")
            bev_b = b16v("bev")
            brv_b = b16v("brv")
            pidx = bfc[:, BF_OFF["pidx"][0]:BF_OFF["pidx"][0] + 1]

            xpool = tc.alloc_tile_pool(name="xchunk", bufs=7)
            xppool = tc.alloc_tile_pool(name="xchunkp", bufs=2)
            prodpool = tc.alloc_tile_pool(name="prod", bufs=4)
            diagpool = tc.alloc_tile_pool(name="diag", bufs=8)
            junkpool = tc.alloc_tile_pool(name="junk", bufs=1)
            tailp = tc.alloc_tile_pool(name="tail", bufs=1)

            junk_d = junkpool.tile([P, D], BF16, tag="junk_d")
            junk_a = junkpool.tile([P, D], BF16, tag="junk_a")

            from types import SimpleNamespace

            def mchain(ts):
                st = SimpleNamespace()
                st.rows = slice(ts * P, (ts + 1) * P)
                x0_sb = x0_tiles[ts]
                st.r_sb = r_tiles[ts]

                # x0p = x0 + pos_e[0]  (bf16)
                x0p = work.tile([P, D], BF16, tag="x0p")
                nc.vector.tensor_tensor(out=x0p[:], in0=x0_sb[:], in1=pe0_b[:], op=ALU.add)
                # x0pT via XBAR on scalar ring
                x0pT = work.tile([P, DC, P], BF16, tag="x0pT")
                for kc in range(DC):
                    nc.scalar.dma_start_transpose(x0pT[:, kc, :], x0p[:, kc * P:(kc + 1) * P])

                # bias dots: qdots = [q.be_k, q.br_k]*scale  (vk2/bq2 pre-scaled)
                ps_b2 = psum.tile([P, 2], F32, tag="ps")
                for kc in range(DC):
                    nc.tensor.matmul(ps_b2[:], x0pT[:, kc, :], vk2_sb[:, kc, :],
                                     start=(kc == 0), stop=False)
                nc.tensor.matmul(ps_b2[:], onescol[0:1, :], bq2row[0:1, :],
                                 start=False, stop=True)
                st.qdots = work.tile([P, 2], F32, tag="qdots")
                nc.vector.tensor_scalar(out=st.qdots[:], in0=ps_b2[:],
                                        scalar1=1.0, scalar2=None, op0=ALU.mult)

                def qk_side(w_sb, u_row, tag):
                    ps_qk = psum.tile([P, D], F32, tag="ps")
                    for kc in range(DC):
                        nc.tensor.matmul(ps_qk[:], x0pT[:, kc, :], w_sb[:, kc, :],
                                         start=(kc == 0), stop=False)
                    nc.tensor.matmul(ps_qk[:], onescol[0:1, :], u_row[0:1, :],
                                     start=False, stop=True)
                    qk16 = work.tile([P, D], BF16, tag=f"qk_{tag}")
                    nc.scalar.activation(out=qk16[:], in_=ps_qk[:], func=ACTF.Copy)
                    qkT = work.tile([P, DC, P], BF16, tag=f"qkT_{tag}")
                    for kc in range(DC):
                        nc.scalar.dma_start_transpose(qkT[:, kc, :], qk16[:, kc * P:(kc + 1) * P])
                    return qk16, qkT

                st.qk_e, qk_eT = qk_side(wqk_e_sb, ue_row, "e")
                st.qk_r, qk_rT = qk_side(wqk_r_sb, ur_row, "r")

                # positional scores
                ps_se = psum.tile([P, L + 1], F32, tag="ps")
                for kc in range(DC):
                    nc.tensor.matmul(ps_se[0:P, 0:L + 1], qk_eT[:, kc, :], peT_sb[:, kc, :],
                                     start=(kc == 0), stop=(kc == DC - 1))
                st.s_pos_e = work.tile([P, L + 1], F32, tag="s_pos_e")
                nc.vector.tensor_scalar(out=st.s_pos_e[:], in0=ps_se[0:P, 0:L + 1],
                                        scalar1=st.qdots[:, 0:1], scalar2=None, op0=ALU.add)

                ps_sr = psum.tile([P, L], F32, tag="ps")
                for kc in range(DC):
                    nc.tensor.matmul(ps_sr[0:P, 0:L], qk_rT[:, kc, :], prT_sb[:, kc, :],
                                     start=(kc == 0), stop=(kc == DC - 1))
                # rdot = r . qk_r
                rdot = work.tile([P, 1], F32, tag="rdot")
                nc.vector.scalar_tensor_tensor(
                    out=junk_d[:], in0=st.r_sb[:], scalar=1.0, in1=st.qk_r[:],
                    op0=ALU.bypass, op1=ALU.mult, accum_out=rdot[:])
                s_r = work.tile([P, L], F32, tag="s_r")
                nc.vector.tensor_scalar(out=s_r[:], in0=ps_sr[0:P, 0:L],
                                        scalar1=st.qdots[:, 1:2], scalar2=rdot[:],
                                        op0=ALU.add, op1=ALU.add)

                # p holds exp(scores): [0:64] entity, 64 mask, 65:129 rel
                st.p = work.tile([P, 2 * L + 1], F32, tag="p")
                nc.scalar.activation(out=st.p[:, L:L + 1], in_=st.s_pos_e[:, L:L + 1], func=ACTF.Exp)
                nc.scalar.activation(out=st.p[:, L + 1:2 * L + 1], in_=s_r[:], func=ACTF.Exp)

                st.sx = work.tile([P, L], F32, tag="sx")
                st.s_ent = work.tile([P, L], F32, tag="s_ent")
                st.O_ps = opsum.tile([P, D], F32, tag="O")

                # --- rel/C_r side: independent of the x stream, run it now ---
                st.sar = work.tile([P, 1], F32, tag="sar")
                nc.vector.tensor_reduce(out=st.sar[:], in_=st.p[:, L + 1:2 * L + 1],
                                        axis=AX.X, op=ALU.add)
                p16r = work.tile([P, P], BF16, tag="p16r")
                nc.vector.memset(p16r[:, L:P], 0.0)
                nc.vector.tensor_copy(p16r[:, 0:L], st.p[:, L + 1:2 * L + 1])
                pTr = work.tile([P, P], BF16, tag="pTr")
                nc.scalar.dma_start_transpose(pTr[:, :], p16r[:, :])
                ps_cr = psum.tile([P, D], F32, tag="ps")
                nc.tensor.matmul(ps_cr[:], pTr[0:L, :], pr_sb[0:L, :],
                                 start=True, stop=True)
                C_r = work.tile([P, D], BF16, tag="C_r")
                nc.vector.scalar_tensor_tensor(
                    out=C_r[:], in0=st.r_sb[:], scalar=st.sar[:], in1=ps_cr[:],
                    op0=ALU.mult, op1=ALU.add)
                st.C_rT = work.tile([P, DC, P], BF16, tag="C_rT")
                for kc in range(DC):
                    nc.scalar.dma_start_transpose(st.C_rT[:, kc, :], C_r[:, kc * P:(kc + 1) * P])
                return st

            def qk_bcast(st):
                return st.qk_e[:].rearrange("p (a b) -> p a b", a=1).broadcast_to([P, KC, D])

            def load_chunk(ts, c):
                if POOL_CHUNK[c]:
                    xc = xppool.tile([P, KC, D], BF16, tag="xcp")
                    nc.gpsimd.dma_start(xc[:], x_d[st_rows[ts], c * KC:(c + 1) * KC, :])
                else:
                    xc = xpool.tile([P, KC, D], BF16, tag="xc")
                    nc.sync.dma_start(xc[:], x_d[st_rows[ts], c * KC:(c + 1) * KC, :])
                return xc

            def emit_mult(ts, st, c, xc):
                prod = prodpool.tile([P, KC, D], BF16, tag="prod")
                eng = nc.gpsimd if POOL_MULT[c] else nc.vector
                eng.tensor_tensor(out=prod[:], in0=xc[:], in1=qk_bcast(st), op=ALU.mult)
                return prod

            def emit_reduces(ts, st, c, prod):
                for kk in range(KC):
                    k = c * KC + kk
                    if kk < KC - ACT_RED_PER_8:
                        nc.vector.tensor_scalar(
                            out=junk_d[:], in0=prod[:, kk, :], scalar1=1.0, scalar2=0.0,
                            op0=ALU.mult, op1=ALU.add, accum_out=st.sx[:, k:k + 1])
                    else:
                        nc.scalar.activation(
                            out=junk_a[:], in_=prod[:, kk, :], func=ACTF.Copy,
                            accum_out=st.sx[:, k:k + 1])
                sl = slice(c * KC, (c + 1) * KC)
                nc.vector.tensor_tensor(out=st.s_ent[:, sl], in0=st.sx[:, sl],
                                        in1=st.s_pos_e[:, sl], op=ALU.add)
                nc.scalar.activation(out=st.p[:, sl], in_=st.s_ent[:, sl], func=ACTF.Exp)

            def emit_updates(ts, st, c, xc):
                for kk in range(KC):
                    k = c * KC + kk
                    dg = diagpool.tile([P, P], BF16, tag="dg")
                    eng = nc.gpsimd if kk < POOL_DIAG_PER_8 else nc.vector
                    eng.tensor_scalar(out=dg[:], in0=ident16[:],
                                      scalar1=st.p[:, k:k + 1], scalar2=None,
                                      op0=ALU.mult)
                    nc.tensor.matmul(st.O_ps[:], dg[:], xc[:, kk, :],
                                     start=(c == 0 and kk == 0), stop=False)

            def stream(ts, st):
                # prefetch the pool-ring chunks first (pool queue is idle-ish
                # during the other tile's head)
                xcs = {}
                for c in range(NCHUNK):
                    if POOL_CHUNK[c]:
                        xcs[c] = load_chunk(ts, c)
                prev = None
                for c in range(NCHUNK):
                    if c not in xcs:
                        xcs[c] = load_chunk(ts, c)
                    prod = emit_mult(ts, st, c, xcs[c])
                    emit_reduces(ts, st, c, prod)
                    # one-chunk skew: updates for chunk c-1 land after chunk c's
                    # mult/reduces are queued, so DVE/pool never stall on exp(c)
                    if prev is not None:
                        emit_updates(ts, st, prev[0], prev[1])
                    prev = (c, xcs[c])
                emit_updates(ts, st, prev[0], prev[1])

            def tail(ts, st):
                sae = tailp.tile([P, 1], F32, tag="sae")
                zr = tailp.tile([P, 1], F32, tag="zr")
                zz = tailp.tile([P, 1], F32, tag="zz")
                nc.vector.tensor_reduce(out=sae[:], in_=st.p[:, 0:L + 1], axis=AX.X, op=ALU.add)
                nc.vector.tensor_tensor(out=zz[:], in0=sae[:], in1=st.sar[:], op=ALU.add)
                nc.vector.reciprocal(zr[:], zz[:])

                # entity+mask weights transposed: copy p[:, 0:128] (cols 65..127
                # are rel values, harmless; only rows 0..64 of the transpose
                # are consumed).
                p16e = tailp.tile([P, P], BF16, tag="p16e")
                nc.vector.tensor_copy(p16e[:], st.p[:, 0:P])
                pTe = tailp.tile([P, P], BF16, tag="pTe")
                nc.sync.dma_start_transpose(pTe[:, :], p16e[:, :])

                # C_e = O_ps + p_e @ P_e  (accumulate positional into the O bank)
                nc.tensor.matmul(st.O_ps[:], pTe[0:L + 1, :], pe_sb[0:L + 1, :],
                                 start=False, stop=True)
                C_e = tailp.tile([P, D], BF16, tag="C_e")
                nc.scalar.activation(out=C_e[:], in_=st.O_ps[:], func=ACTF.Copy)

                C_eT = tailp.tile([P, DC, P], BF16, tag="C_eT")
                for kc in range(DC):
                    nc.sync.dma_start_transpose(C_eT[:, kc, :], C_e[:, kc * P:(kc + 1) * P])
                ps_out = psum.tile([P, D], F32, tag="ps")
                for kc in range(DC):
                    nc.tensor.matmul(ps_out[:], st.C_rT[:, kc, :], wrvT_sb[:, kc, :],
                                     start=(kc == 0), stop=False)
                for kc in range(DC):
                    nc.tensor.matmul(ps_out[:], C_eT[:, kc, :], wevT_sb[:, kc, :],
                                     start=False, stop=(kc == DC - 1))

                tmp1 = tailp.tile([P, D], F32, tag="tmp1")
                nc.vector.scalar_tensor_tensor(
                    out=tmp1[:], in0=bev_b[:], scalar=sae[:], in1=ps_out[:],
                    op0=ALU.mult, op1=ALU.add)
                tmp2 = tailp.tile([P, D], F32, tag="tmp2")
                nc.vector.scalar_tensor_tensor(
                    out=tmp2[:], in0=brv_b[:], scalar=st.sar[:], in1=tmp1[:],
                    op0=ALU.mult, op1=ALU.add)
                out_sb = tailp.tile([P, D], F32, tag="out_sb")
                nc.vector.tensor_scalar(
                    out=out_sb[:], in0=tmp2[:], scalar1=zr[:], scalar2=None, op0=ALU.mult)
                nc.scalar.dma_start(out_d[st.rows, :], out_sb[:])

            st_rows = [slice(ts * P, (ts + 1) * P) for ts in range(NT)]
            states = [mchain(ts) for ts in range(NT)]
            stream(0, states[0])
            # tail-only constants ride the SP ring between the two streams
            nc.sync.dma_start(b16[:, splitB:], b16_d[:, splitB:])
            stream(1, states[1])
            for ts in range(NT):
                tail(ts, states[ts])

            for _pool in (tailp, junkpool, diagpool, prodpool, xppool, xpool):
                _pool.release()

    nc.finalize()
    return nc


def pack_constants(inputs):
    """Host-side folds + layout transforms of the small replicated constants."""
    def arr(name):
        return np.ascontiguousarray(np.asarray(inputs[name], dtype=np.float32))

    def chunked_rows(w):
        # [R, C] -> [128, R//128, C] with element (p, c, j) = w[c*128+p, j]
        r, c = w.shape
        return np.ascontiguousarray(w.reshape(r // P, P, c).transpose(1, 0, 2))

    pos_e = arr("pos_e")
    pos_r = arr("pos_r")
    mask = arr("mask_emb")
    P_e = np.concatenate([pos_e[:L], (pos_e[L] + mask[0])[None, :]], axis=0)  # [65, D]
    P_r = pos_r[:L]

    def chunked_rows_T(m):
        # m: [K, D] -> transpose [D, K] -> [128, DC, K]
        mt = np.ascontiguousarray(m.T)  # [D, K]
        return np.ascontiguousarray(mt.reshape(DC, P, mt.shape[1]).transpose(1, 0, 2))

    def pad_rows(m):
        out = np.zeros((P, m.shape[1]), np.float32)
        out[:m.shape[0]] = m
        return out

    weq = arr("We_q").astype(np.float64)
    wek_ = arr("We_k").astype(np.float64)
    wrk_ = arr("Wr_k").astype(np.float64)
    beq = arr("be_q").astype(np.float64)
    bek = arr("be_k").astype(np.float64)
    brk = arr("br_k").astype(np.float64)
    # fold q projection and SCALE into the score projections:
    #   qk = ((x0p @ We_q^T + be_q) @ W_k) * scale
    wqk_e = (weq.T @ wek_ * SCALE).astype(np.float32)
    wqk_r = (weq.T @ wrk_ * SCALE).astype(np.float32)
    ue_s = ((beq @ wek_) * SCALE).astype(np.float32)
    ur_s = ((beq @ wrk_) * SCALE).astype(np.float32)
    vk = (weq.T @ bek * SCALE).astype(np.float32)
    vr = (weq.T @ brk * SCALE).astype(np.float32)
    bq2row = np.zeros((P, 2), np.float32)
    bq2row[0, 0] = float(beq @ bek) * SCALE
    bq2row[0, 1] = float(beq @ brk) * SCALE
    onescol = np.zeros((P, P), np.float32)
    onescol[0, :] = 1.0

    f16 = {
        "pe0": np.broadcast_to(pos_e[0], (P, D)),
        "wqk_e": chunked_rows(wqk_e),
        "wqk_r": chunked_rows(wqk_r),
        "ue_row": pad_rows(ue_s[None, :]),
        "ur_row": pad_rows(ur_s[None, :]),
        "onescol": onescol,
        "peT": chunked_rows_T(P_e),
        "prT": chunked_rows_T(P_r),
        "vk2": np.stack([vk, vr], 1).reshape(DC, P, 2).transpose(1, 0, 2),
        "bq2row": bq2row,
        "ident": np.eye(P, dtype=np.float32),
        "wevT": chunked_rows(np.ascontiguousarray(arr("We_v").T)),
        "wrvT": chunked_rows(np.ascontiguousarray(arr("Wr_v").T)),
        "pe": pad_rows(P_e),
        "pr": pad_rows(P_r),
        "bev": np.broadcast_to(arr("be_v"), (P, D)),
        "brv": np.broadcast_to(arr("br_v"), (P, D)),
    }
    ffl = {
        "pidx": np.arange(P, dtype=np.float32)[:, None],
    }

    b16 = np.zeros((P, BLOB16_W), BF)
    for name, (off, w) in B16_OFF.items():
        b16[:, off:off + w] = f16[name].reshape(P, w).astype(BF)
    bf = np.zeros((P, BLOBF_W), np.float32)
    for name, (off, w) in BF_OFF.items():
        bf[:, off:off + w] = ffl[name].reshape(P, w)
    return {"c_blob16": b16, "c_blobf": bf}


_STATE = {}


def kernel(**inputs):
    if "nc" not in _STATE:
        _STATE["nc"] = build_nc()
    nc = _STATE["nc"]

    x = np.asarray(inputs["query_entity_encoding"], dtype=np.float32).astype(BF)
    r = np.asarray(inputs["relation_encoding"], dtype=np.float32).astype(BF)
    shared = pack_constants(inputs)

    in_maps = []
    for i in range(NCORES):
        sl = slice(i * BS, (i + 1) * BS)
        m = {"x16": np.ascontiguousarray(x[sl]), "r16": np.ascontiguousarray(r[sl])}
        m.update(shared)
        in_maps.append(m)

    res = run_bass_kernel_spmd(nc, in_maps, list(range(NCORES)))
    out = np.concatenate([res.results[i]["out"] for i in range(NCORES)], axis=0)
    return out


# revision 25
# speedup vs baseline: 1.4766x; 1.4766x over previous
"""Trainium2 Bass kernel for DualHeterogeneousTransformer (returns out[:, 0] only).

Algebraic reduction (reference returns only query row 0):
  q      = (x[:,0,:] + pos_e[0]) @ We_q^T + be_q                      [B,D]
  qk_e   = (q @ We_k) * scale ; qk_r = (q @ Wr_k) * scale             [B,D]
  s_e[b,k] = x[b,k,:].qk_e[b] + P_e[k].qk_e[b] + (q.be_k)*scale   k<64
  s_e[b,64] = P_e[64].qk_e[b] + (q.be_k)*scale    (mask token, P_e[64]=pos_e[64]+mask)
  s_r[b,k] = r[b].qk_r[b] + pos_r[k].qk_r[b] + (q.br_k)*scale
  p = exp(s), Z = sum(p)
  C_e[b,:] = sum_{k<64} p_e[b,k] x[b,k,:]  +  p_e[b,:65] @ P_e
  C_r[b,:] = (sum_k p_r[b,k]) * r[b]  +  p_r @ pos_r[:64]
  out = (C_e @ We_v^T + C_r @ Wr_v^T + sae*be_v + sar*br_v) / Z

Implementation notes (cost-model driven):
  - x and r are converted to bf16 on the host: halves all x DMA bytes.
  - x streams in [128, 8, 512] chunks: most on the SP HWDGE ring (free-standing
    queue), a few via the gpsimd SWDGE ring.
  - dot products: one broadcast tensor_tensor multiply per chunk (DVE 2x mode /
    gpsimd flat) + per-key accumulating reduces split DVE (4x ts-accum) / ACT
    (copy with accum).
  - O-update: diag(p_k) = ident16 * p_k via one tensor_scalar, then PE
    accumulates p_k*x_k into PSUM via diagonal matmuls at ~215ns/key.
  - all transposes via DMA XBAR (dma_start_transpose, bf16) on HWDGE rings.
  - weights/positional tables pre-folded on host (SCALE and We_q folded in),
    shipped as one bf16 blob + tiny f32 blob.
"""

import os
import sys

import numpy as np
import ml_dtypes

for _p in ("/opt/trn_rl_repo", "/root/.axon_site/_ro/trn_rl_repo"):
    if os.path.isdir(_p) and _p not in sys.path:
        sys.path.insert(0, _p)

import concourse.bass as bass
import concourse.bacc as bacc
import concourse.mybir as mybir
from concourse import tile
from concourse.bass_utils import run_bass_kernel_spmd

B, L, D = 2048, 64, 512
NCORES = 8
BS = B // NCORES          # 256 rows per core
P = 128                   # partition tile of batch rows
NT = BS // P              # 2 batch tiles per core
KC = 8                    # keys per streamed x chunk
NCHUNK = L // KC          # 8 chunks per batch tile
DC = D // P               # 4 contraction chunks of 128
SCALE = float(1.0 / np.sqrt(D))
F32 = mybir.dt.float32
BF16 = mybir.dt.bfloat16
ALU = mybir.AluOpType
ACTF = mybir.ActivationFunctionType
AX = mybir.AxisListType
BF = ml_dtypes.bfloat16

# --- tuning knobs -----------------------------------------------------------
# per-tile chunk ring: True -> gpsimd SWDGE, False -> SP HWDGE
POOL_CHUNK = [False, False, False, False, False, False, False, True]
# per-tile chunk mult engine: True -> gpsimd, False -> DVE
POOL_MULT = [False, True, False, True, False, True, False, True]
# per 8 keys: how many reduces go to ACT (rest DVE ts-accum)
ACT_RED_PER_8 = 3
# per 8 keys: how many diag builds go to gpsimd (rest DVE)
POOL_DIAG_PER_8 = 6

# bf16 blob layout: name -> width (free floats per partition)
_B16_FIELDS = [
    # piece A: gates the q-chain start
    ("pe0", D), ("ident", P), ("wqk_e", DC * D), ("ue_row", D), ("onescol", P),
    # piece B: rest of the head
    ("wqk_r", D * DC), ("ur_row", D), ("peT", DC * (L + 1)), ("prT", DC * L),
    ("vk2", DC * 2), ("bq2row", 2), ("pr", D),
    # piece C: tail-only
    ("wevT", DC * D), ("wrvT", DC * D), ("pe", D),
    ("bev", D), ("brv", D),
]
_PIECE_A_END = "wqk_r"
_PIECE_B_END = "wevT"

_BF_FIELDS = [("pidx", 1)]


def _offsets(fields):
    out, off = {}, 0
    for name, w in fields:
        out[name] = (off, w)
        off += w
    return out, off


B16_OFF, BLOB16_W = _offsets(_B16_FIELDS)
BF_OFF, BLOBF_W = _offsets(_BF_FIELDS)


def build_nc():
    nc = bacc.Bacc("TRN2", target_bir_lowering=False, debug=False)

    x_d = nc.dram_tensor("x16", [BS, L, D], BF16, kind="ExternalInput")
    r_d = nc.dram_tensor("r16", [BS, D], BF16, kind="ExternalInput")
    b16_d = nc.dram_tensor("c_blob16", [P, BLOB16_W], BF16, kind="ExternalInput")
    bf_d = nc.dram_tensor("c_blobf", [P, BLOBF_W], F32, kind="ExternalInput")
    out_d = nc.dram_tensor("out", [BS, D], F32, kind="ExternalOutput")

    with tile.TileContext(nc) as tc:
        with (
            tc.tile_pool(name="const", bufs=1) as const,
            tc.tile_pool(name="work", bufs=2) as work,
            tc.tile_pool(name="psum", bufs=3, space="PSUM") as psum,
            tc.tile_pool(name="psumt", bufs=2, space="PSUM") as psumt,
            tc.tile_pool(name="opsum", bufs=2, space="PSUM") as opsum,
        ):
            # PE warmup to ramp the HAM clock before the q-chain matmuls.
            warm = work.tile([P, P], BF16, tag="warm")
            nc.vector.memset(warm[:], 0.0)
            ps_w = psum.tile([P, P], F32, tag="ps")
            for wi in range(10):
                nc.tensor.matmul(ps_w[:], warm[:], warm[:],
                                 start=(wi == 0), stop=(wi == 9))

            # head DMAs: x0 rows (tiny, SP), r rows (SP), blobs (ACT ring)
            x0_tiles, r_tiles = [], []
            for ts in range(NT):
                rows = slice(ts * P, (ts + 1) * P)
                x0_t = work.tile([P, D], BF16, tag="x0")
                nc.sync.dma_start(x0_t[:], x_d[rows, 0, :])
                x0_tiles.append(x0_t)

            b16 = const.tile([P, BLOB16_W], BF16, tag="b16")
            splitA = B16_OFF[_PIECE_A_END][0]
            splitB = B16_OFF[_PIECE_B_END][0]
            nc.scalar.dma_start(b16[:, 0:splitA], b16_d[:, 0:splitA])
            nc.scalar.dma_start(b16[:, splitA:splitB], b16_d[:, splitA:splitB])
            # tail-only constants ride the SWDGE lanes (separate from HWDGE)
            nc.gpsimd.dma_start(b16[:, splitB:], b16_d[:, splitB:])
            bfc = const.tile([P, BLOBF_W], F32, tag="bf")
            nc.scalar.dma_start(bfc[:], bf_d[:])

            for ts in range(NT):
                rows = slice(ts * P, (ts + 1) * P)
                r_t = work.tile([P, D], BF16, tag="r")
                nc.sync.dma_start(r_t[:], r_d[rows, :])
                r_tiles.append(r_t)

            def b16v(name, *dims):
                off, w = B16_OFF[name]
                v = b16[:, off:off + w]
                if dims:
                    kw = {chr(97 + i): d_ for i, d_ in enumerate(dims)}
                    pat = " ".join(chr(97 + i) for i in range(len(dims)))
                    v = v.rearrange(f"p ({pat}) -> p {pat}", **kw)
                return v

            pe0_b = b16v("pe0")
            wqk_e_sb = b16v("wqk_e", DC, D)
            wqk_r_sb = b16v("wqk_r", DC, D)
            ue_row = b16v("ue_row")
            ur_row = b16v("ur_row")
            onescol = b16v("onescol")
            peT_sb = b16v("peT", DC, L + 1)
            prT_sb = b16v("prT", DC, L)
            vk2_sb = b16v("vk2", DC, 2)
            bq2row = b16v("bq2row")
            ident16 = b16v("ident")
            wevT_sb = b16v("wevT", DC, D)
            wrvT_sb = b16v("wrvT", DC, D)
            pe_sb = b16v("pe")
            pr_sb = b16v("pr")
            bev_b = b16v("bev")
            brv_b = b16v("brv")
            pidx = bfc[:, BF_OFF["pidx"][0]:BF_OFF["pidx"][0] + 1]

            xpool = tc.alloc_tile_pool(name="xchunk", bufs=7)
            xppool = tc.alloc_tile_pool(name="xchunkp", bufs=2)
            prodpool = tc.alloc_tile_pool(name="prod", bufs=4)
            diagpool = tc.alloc_tile_pool(name="diag", bufs=8)
            junkpool = tc.alloc_tile_pool(name="junk", bufs=1)
            tailp = tc.alloc_tile_pool(name="tail", bufs=1)

            junk_d = junkpool.tile([P, D], BF16, tag="junk_d")
            junk_a = junkpool.tile([P, D], BF16, tag="junk_a")

            from types import SimpleNamespace

            def mchain(ts):
                st = SimpleNamespace()
                st.rows = slice(ts * P, (ts + 1) * P)
                x0_sb = x0_tiles[ts]
                st.r_sb = r_tiles[ts]

                # x0p = x0 + pos_e[0]  (bf16)
                x0p = work.tile([P, D], BF16, tag="x0p")
                nc.vector.tensor_tensor(out=x0p[:], in0=x0_sb[:], in1=pe0_b[:], op=ALU.add)
                # x0pT on PE (keeps the HWDGE DMA lanes free for x chunks)
                x0pT = work.tile([P, DC, P], BF16, tag="x0pT")
                ps_t0 = psumt.tile([P, DC, P], BF16, tag="pst")
                for kc in range(DC):
                    nc.tensor.transpose(ps_t0[:, kc, :], x0p[:, kc * P:(kc + 1) * P], ident16[:])
                nc.scalar.activation(out=x0pT[:].rearrange("p a b -> p (a b)"),
                                     in_=ps_t0[:].rearrange("p a b -> p (a b)"),
                                     func=ACTF.Copy)

                # bias dots: qdots = [q.be_k, q.br_k]*scale  (vk2/bq2 pre-scaled)
                ps_b2 = psum.tile([P, 2], F32, tag="ps")
                for kc in range(DC):
                    nc.tensor.matmul(ps_b2[:], x0pT[:, kc, :], vk2_sb[:, kc, :],
                                     start=(kc == 0), stop=False)
                nc.tensor.matmul(ps_b2[:], onescol[0:1, :], bq2row[0:1, :],
                                 start=False, stop=True)
                st.qdots = work.tile([P, 2], F32, tag="qdots")
                nc.vector.tensor_scalar(out=st.qdots[:], in0=ps_b2[:],
                                        scalar1=1.0, scalar2=None, op0=ALU.mult)

                def qk_side(w_sb, u_row, tag):
                    ps_qk = psum.tile([P, D], F32, tag="ps")
                    for kc in range(DC):
                        nc.tensor.matmul(ps_qk[:], x0pT[:, kc, :], w_sb[:, kc, :],
                                         start=(kc == 0), stop=False)
                    nc.tensor.matmul(ps_qk[:], onescol[0:1, :], u_row[0:1, :],
                                     start=False, stop=True)
                    qk16 = work.tile([P, D], BF16, tag=f"qk_{tag}")
                    nc.scalar.activation(out=qk16[:], in_=ps_qk[:], func=ACTF.Copy)
                    qkT = work.tile([P, DC, P], BF16, tag=f"qkT_{tag}")
                    ps_t = psumt.tile([P, DC, P], BF16, tag="pst")
                    for kc in range(DC):
                        nc.tensor.transpose(ps_t[:, kc, :], qk16[:, kc * P:(kc + 1) * P], ident16[:])
                    nc.scalar.activation(out=qkT[:].rearrange("p a b -> p (a b)"),
                                         in_=ps_t[:].rearrange("p a b -> p (a b)"),
                                         func=ACTF.Copy)
                    return qk16, qkT

                st.qk_e, qk_eT = qk_side(wqk_e_sb, ue_row, "e")
                st.qk_r, qk_rT = qk_side(wqk_r_sb, ur_row, "r")

                # positional scores
                ps_se = psum.tile([P, L + 1], F32, tag="ps")
                for kc in range(DC):
                    nc.tensor.matmul(ps_se[0:P, 0:L + 1], qk_eT[:, kc, :], peT_sb[:, kc, :],
                                     start=(kc == 0), stop=(kc == DC - 1))
                st.s_pos_e = work.tile([P, L + 1], F32, tag="s_pos_e")
                nc.vector.tensor_scalar(out=st.s_pos_e[:], in0=ps_se[0:P, 0:L + 1],
                                        scalar1=st.qdots[:, 0:1], scalar2=None, op0=ALU.add)

                ps_sr = psum.tile([P, L], F32, tag="ps")
                for kc in range(DC):
                    nc.tensor.matmul(ps_sr[0:P, 0:L], qk_rT[:, kc, :], prT_sb[:, kc, :],
                                     start=(kc == 0), stop=(kc == DC - 1))
                # rdot = r . qk_r
                rdot = work.tile([P, 1], F32, tag="rdot")
                nc.vector.scalar_tensor_tensor(
                    out=junk_d[:], in0=st.r_sb[:], scalar=1.0, in1=st.qk_r[:],
                    op0=ALU.bypass, op1=ALU.mult, accum_out=rdot[:])
                s_r = work.tile([P, L], F32, tag="s_r")
                nc.vector.tensor_scalar(out=s_r[:], in0=ps_sr[0:P, 0:L],
                                        scalar1=st.qdots[:, 1:2], scalar2=rdot[:],
                                        op0=ALU.add, op1=ALU.add)

                # p holds exp(scores): [0:64] entity, 64 mask, 65:129 rel
                st.p = work.tile([P, 2 * L + 1], F32, tag="p")
                nc.scalar.activation(out=st.p[:, L:L + 1], in_=st.s_pos_e[:, L:L + 1], func=ACTF.Exp)
                nc.scalar.activation(out=st.p[:, L + 1:2 * L + 1], in_=s_r[:], func=ACTF.Exp)

                st.sx = work.tile([P, L], F32, tag="sx")
                st.s_ent = work.tile([P, L], F32, tag="s_ent")
                st.O_ps = opsum.tile([P, D], F32, tag="O")

                return st

            def mchain2(ts, st):
                # rel/C_r side: independent of the x stream; its XBARs ride the
                # SP ring AFTER the chunk DMAs (DMA-lane ticks stay behind the
                # chunks, and SP is free again by then).
                st.sar = work.tile([P, 1], F32, tag="sar")
                nc.vector.tensor_reduce(out=st.sar[:], in_=st.p[:, L + 1:2 * L + 1],
                                        axis=AX.X, op=ALU.add)
                p16r = work.tile([P, P], BF16, tag="p16r")
                nc.vector.memset(p16r[:, L:P], 0.0)
                nc.vector.tensor_copy(p16r[:, 0:L], st.p[:, L + 1:2 * L + 1])
                pTr = work.tile([P, P], BF16, tag="pTr")
                nc.sync.dma_start_transpose(pTr[:, :], p16r[:, :])
                ps_cr = psum.tile([P, D], F32, tag="ps")
                nc.tensor.matmul(ps_cr[:], pTr[0:L, :], pr_sb[0:L, :],
                                 start=True, stop=True)
                C_r = work.tile([P, D], BF16, tag="C_r")
                nc.vector.scalar_tensor_tensor(
                    out=C_r[:], in0=st.r_sb[:], scalar=st.sar[:], in1=ps_cr[:],
                    op0=ALU.mult, op1=ALU.add)
                st.C_rT = work.tile([P, DC, P], BF16, tag="C_rT")
                for kc in range(DC):
                    nc.sync.dma_start_transpose(st.C_rT[:, kc, :], C_r[:, kc * P:(kc + 1) * P])

            def qk_bcast(st):
                return st.qk_e[:].rearrange("p (a b) -> p a b", a=1).broadcast_to([P, KC, D])

            def load_chunk(ts, c):
                if POOL_CHUNK[c]:
                    xc = xppool.tile([P, KC, D], BF16, tag="xcp")
                    nc.gpsimd.dma_start(xc[:], x_d[st_rows[ts], c * KC:(c + 1) * KC, :])
                else:
                    xc = xpool.tile([P, KC, D], BF16, tag="xc")
                    nc.sync.dma_start(xc[:], x_d[st_rows[ts], c * KC:(c + 1) * KC, :])
                return xc

            def emit_mult(ts, st, c, xc):
                prod = prodpool.tile([P, KC, D], BF16, tag="prod")
                eng = nc.gpsimd if POOL_MULT[c] else nc.vector
                eng.tensor_tensor(out=prod[:], in0=xc[:], in1=qk_bcast(st), op=ALU.mult)
                return prod

            def emit_reduces(ts, st, c, prod):
                for kk in range(KC):
                    k = c * KC + kk
                    if kk < KC - ACT_RED_PER_8:
                        nc.vector.tensor_scalar(
                            out=junk_d[:], in0=prod[:, kk, :], scalar1=1.0, scalar2=0.0,
                            op0=ALU.mult, op1=ALU.add, accum_out=st.sx[:, k:k + 1])
                    else:
                        nc.scalar.activation(
                            out=junk_a[:], in_=prod[:, kk, :], func=ACTF.Copy,
                            accum_out=st.sx[:, k:k + 1])
                sl = slice(c * KC, (c + 1) * KC)
                nc.vector.tensor_tensor(out=st.s_ent[:, sl], in0=st.sx[:, sl],
                                        in1=st.s_pos_e[:, sl], op=ALU.add)
                nc.scalar.activation(out=st.p[:, sl], in_=st.s_ent[:, sl], func=ACTF.Exp)

            def emit_updates(ts, st, c, xc):
                for kk in range(KC):
                    k = c * KC + kk
                    dg = diagpool.tile([P, P], BF16, tag="dg")
                    eng = nc.gpsimd if kk < POOL_DIAG_PER_8 else nc.vector
                    eng.tensor_scalar(out=dg[:], in0=ident16[:],
                                      scalar1=st.p[:, k:k + 1], scalar2=None,
                                      op0=ALU.mult)
                    nc.tensor.matmul(st.O_ps[:], dg[:], xc[:, kk, :],
                                     start=(c == 0 and kk == 0), stop=False)

            def stream(ts, st):
                xcs = xcs_all[ts]
                prev = None
                for c in range(NCHUNK):
                    prod = emit_mult(ts, st, c, xcs[c])
                    emit_reduces(ts, st, c, prod)
                    # one-chunk skew: updates for chunk c-1 land after chunk c's
                    # mult/reduces are queued, so DVE/pool never stall on exp(c)
                    if prev is not None:
                        emit_updates(ts, st, prev[0], prev[1])
                    prev = (c, xcs[c])
                emit_updates(ts, st, prev[0], prev[1])

            def tail(ts, st):
                sae = tailp.tile([P, 1], F32, tag="sae")
                zr = tailp.tile([P, 1], F32, tag="zr")
                zz = tailp.tile([P, 1], F32, tag="zz")
                nc.vector.tensor_reduce(out=sae[:], in_=st.p[:, 0:L + 1], axis=AX.X, op=ALU.add)
                nc.vector.tensor_tensor(out=zz[:], in0=sae[:], in1=st.sar[:], op=ALU.add)
                nc.vector.reciprocal(zr[:], zz[:])

                # entity+mask weights transposed: copy p[:, 0:128] (cols 65..127
                # are rel values, harmless; only rows 0..64 of the transpose
                # are consumed).
                p16e = tailp.tile([P, P], BF16, tag="p16e")
                nc.vector.tensor_copy(p16e[:], st.p[:, 0:P])
                pTe = tailp.tile([P, P], BF16, tag="pTe")
                nc.sync.dma_start_transpose(pTe[:, :], p16e[:, :])

                # C_e = O_ps + p_e @ P_e  (accumulate positional into the O bank)
                nc.tensor.matmul(st.O_ps[:], pTe[0:L + 1, :], pe_sb[0:L + 1, :],
                                 start=False, stop=True)
                C_e = tailp.tile([P, D], BF16, tag="C_e")
                nc.scalar.activation(out=C_e[:], in_=st.O_ps[:], func=ACTF.Copy)

                C_eT = tailp.tile([P, DC, P], BF16, tag="C_eT")
                for kc in range(DC):
                    nc.sync.dma_start_transpose(C_eT[:, kc, :], C_e[:, kc * P:(kc + 1) * P])
                ps_out = psum.tile([P, D], F32, tag="ps")
                for kc in range(DC):
                    nc.tensor.matmul(ps_out[:], st.C_rT[:, kc, :], wrvT_sb[:, kc, :],
                                     start=(kc == 0), stop=False)
                for kc in range(DC):
                    nc.tensor.matmul(ps_out[:], C_eT[:, kc, :], wevT_sb[:, kc, :],
                                     start=False, stop=(kc == DC - 1))

                tmp1 = tailp.tile([P, D], F32, tag="tmp1")
                nc.vector.scalar_tensor_tensor(
                    out=tmp1[:], in0=bev_b[:], scalar=sae[:], in1=ps_out[:],
                    op0=ALU.mult, op1=ALU.add)
                tmp2 = tailp.tile([P, D], F32, tag="tmp2")
                nc.vector.scalar_tensor_tensor(
                    out=tmp2[:], in0=brv_b[:], scalar=st.sar[:], in1=tmp1[:],
                    op0=ALU.mult, op1=ALU.add)
                out_sb = tailp.tile([P, D], F32, tag="out_sb")
                nc.vector.tensor_scalar(
                    out=out_sb[:], in0=tmp2[:], scalar1=zr[:], scalar2=None, op0=ALU.mult)
                nc.scalar.dma_start(out_d[st.rows, :], out_sb[:])

            st_rows = [slice(ts * P, (ts + 1) * P) for ts in range(NT)]
            states = [mchain(ts) for ts in range(NT)]
            # all x-chunk DMAs up front: the HWDGE lanes see only chunks, in
            # order, so they stream back-to-back with no cross-waits
            xcs_all = [dict() for _ in range(NT)]
            for ts in range(NT):
                for c in range(NCHUNK):
                    if POOL_CHUNK[c]:
                        xcs_all[ts][c] = load_chunk(ts, c)
            for ts in range(NT):
                for c in range(NCHUNK):
                    if not POOL_CHUNK[c]:
                        xcs_all[ts][c] = load_chunk(ts, c)
            for ts in range(NT):
                mchain2(ts, states[ts])
            stream(0, states[0])
            stream(1, states[1])
            for ts in range(NT):
                tail(ts, states[ts])

            for _pool in (tailp, junkpool, diagpool, prodpool, xppool, xpool):
                _pool.release()

    nc.finalize()
    return nc


def pack_constants(inputs):
    """Host-side folds + layout transforms of the small replicated constants."""
    def arr(name):
        return np.ascontiguousarray(np.asarray(inputs[name], dtype=np.float32))

    def chunked_rows(w):
        # [R, C] -> [128, R//128, C] with element (p, c, j) = w[c*128+p, j]
        r, c = w.shape
        return np.ascontiguousarray(w.reshape(r // P, P, c).transpose(1, 0, 2))

    pos_e = arr("pos_e")
    pos_r = arr("pos_r")
    mask = arr("mask_emb")
    P_e = np.concatenate([pos_e[:L], (pos_e[L] + mask[0])[None, :]], axis=0)  # [65, D]
    P_r = pos_r[:L]

    def chunked_rows_T(m):
        # m: [K, D] -> transpose [D, K] -> [128, DC, K]
        mt = np.ascontiguousarray(m.T)  # [D, K]
        return np.ascontiguousarray(mt.reshape(DC, P, mt.shape[1]).transpose(1, 0, 2))

    def pad_rows(m):
        out = np.zeros((P, m.shape[1]), np.float32)
        out[:m.shape[0]] = m
        return out

    weq = arr("We_q").astype(np.float64)
    wek_ = arr("We_k").astype(np.float64)
    wrk_ = arr("Wr_k").astype(np.float64)
    beq = arr("be_q").astype(np.float64)
    bek = arr("be_k").astype(np.float64)
    brk = arr("br_k").astype(np.float64)
    # fold q projection and SCALE into the score projections:
    #   qk = ((x0p @ We_q^T + be_q) @ W_k) * scale
    wqk_e = (weq.T @ wek_ * SCALE).astype(np.float32)
    wqk_r = (weq.T @ wrk_ * SCALE).astype(np.float32)
    ue_s = ((beq @ wek_) * SCALE).astype(np.float32)
    ur_s = ((beq @ wrk_) * SCALE).astype(np.float32)
    vk = (weq.T @ bek * SCALE).astype(np.float32)
    vr = (weq.T @ brk * SCALE).astype(np.float32)
    bq2row = np.zeros((P, 2), np.float32)
    bq2row[0, 0] = float(beq @ bek) * SCALE
    bq2row[0, 1] = float(beq @ brk) * SCALE
    onescol = np.zeros((P, P), np.float32)
    onescol[0, :] = 1.0

    f16 = {
        "pe0": np.broadcast_to(pos_e[0], (P, D)),
        "wqk_e": chunked_rows(wqk_e),
        "wqk_r": chunked_rows(wqk_r),
        "ue_row": pad_rows(ue_s[None, :]),
        "ur_row": pad_rows(ur_s[None, :]),
        "onescol": onescol,
        "peT": chunked_rows_T(P_e),
        "prT": chunked_rows_T(P_r),
        "vk2": np.stack([vk, vr], 1).reshape(DC, P, 2).transpose(1, 0, 2),
        "bq2row": bq2row,
        "ident": np.eye(P, dtype=np.float32),
        "wevT": chunked_rows(np.ascontiguousarray(arr("We_v").T)),
        "wrvT": chunked_rows(np.ascontiguousarray(arr("Wr_v").T)),
        "pe": pad_rows(P_e),
        "pr": pad_rows(P_r),
        "bev": np.broadcast_to(arr("be_v"), (P, D)),
        "brv": np.broadcast_to(arr("br_v"), (P, D)),
    }
    ffl = {
        "pidx": np.arange(P, dtype=np.float32)[:, None],
    }

    b16 = np.zeros((P, BLOB16_W), BF)
    for name, (off, w) in B16_OFF.items():
        b16[:, off:off + w] = f16[name].reshape(P, w).astype(BF)
    bf = np.zeros((P, BLOBF_W), np.float32)
    for name, (off, w) in BF_OFF.items():
        bf[:, off:off + w] = ffl[name].reshape(P, w)
    return {"c_blob16": b16, "c_blobf": bf}


_STATE = {}


def kernel(**inputs):
    if "nc" not in _STATE:
        _STATE["nc"] = build_nc()
    nc = _STATE["nc"]

    x = np.asarray(inputs["query_entity_encoding"], dtype=np.float32).astype(BF)
    r = np.asarray(inputs["relation_encoding"], dtype=np.float32).astype(BF)
    shared = pack_constants(inputs)

    in_maps = []
    for i in range(NCORES):
        sl = slice(i * BS, (i + 1) * BS)
        m = {"x16": np.ascontiguousarray(x[sl]), "r16": np.ascontiguousarray(r[sl])}
        m.update(shared)
        in_maps.append(m)

    res = run_bass_kernel_spmd(nc, in_maps, list(range(NCORES)))
    out = np.concatenate([res.results[i]["out"] for i in range(NCORES)], axis=0)
    return out
